# revision 2
# baseline (speedup 1.0000x reference)
"""Self-contained Trainium2 Bass kernel for nn_AttentionBlock (GroupNorm +
self/cross attention + projection + residual), data-parallel over batch on
8 NeuronCores.

Contract: kernel(**inputs) takes the FULL unsharded inputs of
reference.setup_inputs() and returns the FULL [16, 512, 32, 32] fp32 output.
"""
import numpy as np
import concourse.bass as bass
import concourse.mybir as mybir
import concourse.tile as tile
from concourse.bass_utils import run_bass_kernel_spmd

# Problem shapes (hardcoded per contract)
B, C, HS, WS = 16, 512, 32, 32
T = HS * WS          # 1024 spatial positions
HEADS, HC = 8, 64
L = 77               # cond sequence
CD = 768             # cond dim
S = T + L            # 1101 total key positions
GROUPS = 32
GSIZE = C // GROUPS  # 16 channels per group
EPS = 1e-5
N_CORES = 8
BPC = B // N_CORES   # batches per core
CT = C // 128        # 4 channel tiles
CCT = CD // 128      # 6 cond channel tiles
NSC = 9              # key-position chunks: 8 x 128 + 77
TH = 2               # t halves of 512

F16 = mybir.dt.float16
F32 = mybir.dt.float32
AF = mybir.ActivationFunctionType
AL = mybir.AluOpType
AX = mybir.AxisListType

_CACHE = {}


def split_multi_waits(nc):
    """walrus in this container accepts at most one sync wait per instruction;
    hoist extra waits onto preceding NOPs on the same engine."""
    n_split = 0
    for f in nc.m.functions:
        for blk in f.blocks:
            new_insts = []
            for inst in blk.instructions:
                si = inst.sync_info
                if si is not None and si.on_wait is not None and len(si.on_wait) > 1:
                    waits = list(si.on_wait)
                    for w in waits[:-1]:
                        nop = mybir.InstNoOp(
                            name=f"{inst.name}-wsplit{n_split}",
                            ins=[], outs=[],
                            sync_info=mybir.SyncInfo(on_wait=[w], on_update=[]),
                        )
                        nop.engine = inst.engine
                        new_insts.append(nop)
                        n_split += 1
                    si.on_wait = [waits[-1]]
                    inst.sync_info = si
                new_insts.append(inst)
            blk.instructions = new_insts
    return n_split


def build_program(apply_vbias=False, apply_pbias=False):
    nc = bass.Bass("TRN2", target_bir_lowering=False, debug=False, num_devices=1)

    xd = nc.dram_tensor("x_sh", [BPC, C, T], F32, kind="ExternalInput")
    cd = nc.dram_tensor("c_sh", [BPC, CD, L], F16, kind="ExternalInput")
    wqd = nc.dram_tensor("wqT", [C, C], F16, kind="ExternalInput")
    wkd = nc.dram_tensor("wkT", [C, C], F16, kind="ExternalInput")
    wvd = nc.dram_tensor("wvT", [C, C], F16, kind="ExternalInput")
    wkcd = nc.dram_tensor("wkcT", [CD, C], F16, kind="ExternalInput")
    wvcd = nc.dram_tensor("wvcT", [CD, C], F16, kind="ExternalInput")
    wpd = nc.dram_tensor("wpT", [C, C], F16, kind="ExternalInput")
    Gd = nc.dram_tensor("G", [128, CT, GROUPS], F32, kind="ExternalInput")
    GTd = nc.dram_tensor("GT", [GROUPS, CT, 128], F32, kind="ExternalInput")
    qbd = nc.dram_tensor("qb", [128, CT], F32, kind="ExternalInput")
    kbd = nc.dram_tensor("kb", [128, CT], F32, kind="ExternalInput")
    kcbd = nc.dram_tensor("kcb", [128, CT], F32, kind="ExternalInput")
    pbd = nc.dram_tensor("pb", [128, CT], F32, kind="ExternalInput")
    vbd = nc.dram_tensor("vbrow", [1, HEADS * 65], F16, kind="ExternalInput")
    outd = nc.dram_tensor("out", [BPC, C, T], F32, kind="ExternalOutput")

    with tile.TileContext(nc) as tc:
        with tc.tile_pool(name="wp", bufs=1) as wp, \
             tc.tile_pool(name="xp", bufs=8) as xp, \
             tc.tile_pool(name="gnp", bufs=8) as gnp, \
             tc.tile_pool(name="qkp", bufs=8) as qkp, \
             tc.tile_pool(name="vtp", bufs=18) as vtp, \
             tc.tile_pool(name="ptp", bufs=6) as ptp, \
             tc.tile_pool(name="app", bufs=8) as app, \
             tc.tile_pool(name="nmp", bufs=3) as nmp, \
             tc.tile_pool(name="psp", bufs=2, space="PSUM") as psp:

            # ---- constants & weights -------------------------------------
            wq_sb = wp.tile([128, CT, C], F16, name="wq_sb")
            wk_sb = wp.tile([128, CT, C], F16, name="wk_sb")
            wv_sb = wp.tile([128, CT, C], F16, name="wv_sb")
            wkc_sb = wp.tile([128, CCT, C], F16, name="wkc_sb")
            wvc_sb = wp.tile([128, CCT, C], F16, name="wvc_sb")
            wp_sb = wp.tile([128, CT, C], F16, name="wp_sb")
            nc.sync.dma_start(wq_sb[:], wqd.ap().rearrange("(a p) o -> p a o", p=128))
            nc.sync.dma_start(wk_sb[:], wkd.ap().rearrange("(a p) o -> p a o", p=128))
            nc.sync.dma_start(wv_sb[:], wvd.ap().rearrange("(a p) o -> p a o", p=128))
            nc.sync.dma_start(wkc_sb[:], wkcd.ap().rearrange("(a p) o -> p a o", p=128))
            nc.sync.dma_start(wvc_sb[:], wvcd.ap().rearrange("(a p) o -> p a o", p=128))
            nc.sync.dma_start(wp_sb[:], wpd.ap().rearrange("(a p) o -> p a o", p=128))
            G_sb = wp.tile([128, CT, GROUPS], F32, name="G_sb")
            GT_sb = wp.tile([GROUPS, CT, 128], F32, name="GT_sb")
            nc.sync.dma_start(G_sb[:], Gd.ap())
            nc.sync.dma_start(GT_sb[:], GTd.ap())
            qb_sb = wp.tile([128, CT], F32, name="qb_sb")
            kb_sb = wp.tile([128, CT], F32, name="kb_sb")
            kcb_sb = wp.tile([128, CT], F32, name="kcb_sb")
            pb_sb = wp.tile([128, CT], F32, name="pb_sb")
            nc.sync.dma_start(qb_sb[:], qbd.ap())
            nc.sync.dma_start(kb_sb[:], kbd.ap())
            nc.sync.dma_start(kcb_sb[:], kcbd.ap())
            nc.sync.dma_start(pb_sb[:], pbd.ap())
            ones16 = wp.tile([65, HC], F16, name="ones16")
            nc.vector.memset(ones16[:], 1.0)
            eps_sb = wp.tile([GROUPS, 1], F32, name="eps_sb")
            nc.vector.memset(eps_sb[:], EPS)
            if apply_vbias:
                vb_bc = wp.tile([128, HEADS * 65], F16, name="vb_bc")
                nc.gpsimd.dma_start(
                    out=vb_bc[:],
                    in_=bass.AP(tensor=vbd.ap().tensor, offset=0,
                                ap=[[0, 128], [1, HEADS * 65]]),
                )

            # ---- input DMAs ----------------------------------------------
            x_sb = {}
            c_sb = {}
            for b in range(BPC):
                for m in range(CT):
                    t_ = xp.tile([128, T], F32, name=f"x_{b}_{m}", tag="x")
                    nc.sync.dma_start(t_[:], xd.ap()[b, 128 * m:128 * (m + 1), :])
                    x_sb[(b, m)] = t_
                t_ = gnp.tile([128, CCT, L], F16, name=f"c_{b}", tag="c")
                nc.sync.dma_start(t_[:], cd.ap()[b].rearrange("(a p) l -> p a l", p=128))
                c_sb[b] = t_

            # ---- GroupNorm stats for both batches (sqrts early, before exps)
            E_sb = {}
            for b in range(BPC):
                s12 = {}
                for m in range(CT):
                    st = gnp.tile([128, 2, 6], F32, name=f"bnst_{b}_{m}", tag="bnst")
                    for sg in range(2):
                        nc.vector.bn_stats(out=st[:, sg, :],
                                           in_=x_sb[(b, m)][:, 512 * sg:512 * (sg + 1)])
                    mv = gnp.tile([128, 2], F32, name=f"mv_{b}_{m}", tag="mv")
                    nc.vector.bn_aggr(out=mv[:], in_=st[:])
                    s12t = gnp.tile([128, 2], F32, name=f"s12_{b}_{m}", tag="s12")
                    # col0 = per-channel mean, col1 = per-channel E[x^2]
                    nc.vector.tensor_copy(s12t[:, 0:1], mv[:, 0:1])
                    nc.vector.tensor_scalar(
                        out=s12t[:, 1:2], in0=mv[:, 0:1], scalar1=mv[:, 0:1],
                        scalar2=None, op0=AL.mult)
                    nc.vector.tensor_tensor(
                        out=s12t[:, 1:2], in0=s12t[:, 1:2], in1=mv[:, 1:2], op=AL.add)
                    s12[m] = s12t
                st_ps = psp.tile([GROUPS, 2], F32, name=f"stps_{b}", tag="a")
                for m in range(CT):
                    nc.tensor.matmul(st_ps[:], G_sb[:, m, :], s12[m][:],
                                     start=(m == 0), stop=(m == CT - 1))
                grp = gnp.tile([GROUPS, 6], F32, name=f"grp_{b}", tag="grp")
                nc.vector.tensor_copy(grp[:, 4:6], st_ps[:])
                # cols: 0=-mean*rstd 1=rstd 2=mean 3=var 4=gsum_mean 5=gsum_e2
                nc.vector.tensor_scalar_mul(grp[:, 2:3], in0=grp[:, 4:5],
                                            scalar1=1.0 / GSIZE)
                nc.vector.tensor_scalar_mul(grp[:, 3:4], in0=grp[:, 5:6],
                                            scalar1=1.0 / GSIZE)
                nc.vector.tensor_scalar(out=grp[:, 1:2], in0=grp[:, 2:3],
                                        scalar1=grp[:, 2:3], scalar2=None, op0=AL.mult)
                nc.vector.tensor_tensor(out=grp[:, 3:4], in0=grp[:, 3:4],
                                        in1=grp[:, 1:2], op=AL.subtract)
                nc.scalar.activation(grp[:, 3:4], grp[:, 3:4], AF.Sqrt, bias=eps_sb[:])
                nc.vector.reciprocal(grp[:, 1:2], grp[:, 3:4])
                nc.vector.tensor_scalar(out=grp[:, 0:1], in0=grp[:, 2:3],
                                        scalar1=grp[:, 1:2], scalar2=-1.0,
                                        op0=AL.mult, op1=AL.mult)
                for m in range(CT):
                    e_ps = psp.tile([128, 2], F32, name=f"eps_{b}_{m}", tag="a")
                    nc.tensor.matmul(e_ps[:], GT_sb[:, m, :], grp[:, 0:2],
                                     start=True, stop=True)
                    e_t = gnp.tile([128, 2], F32, name=f"E_{b}_{m}", tag="E")
                    nc.vector.tensor_copy(e_t[:], e_ps[:])
                    E_sb[(b, m)] = e_t

            # ---- per-batch main pipeline ---------------------------------
            for b in range(BPC):
                # xn = (x - mean) * rstd  (affine folded into weights host-side)
                xn = {}
                for m in range(CT):
                    xnt = qkp.tile([128, T], F16, name=f"xn_{b}_{m}", tag="xn")
                    nc.scalar.activation(xnt[:], x_sb[(b, m)][:], AF.Identity,
                                         bias=E_sb[(b, m)][:, 0:1],
                                         scale=E_sb[(b, m)][:, 1:2])
                    xn[m] = xnt

                # q/k projections: [c_out tile, t]
                q_sb = {}
                k_sb = {}
                for m in range(CT):
                    q_ps = psp.tile([128, T], F32, name=f"qps_{b}_{m}", tag="sc")
                    for th in range(TH):
                        for ci in range(CT):
                            nc.tensor.matmul(
                                q_ps[:, 512 * th:512 * (th + 1)],
                                wq_sb[:, ci, 128 * m:128 * (m + 1)],
                                xn[ci][:, 512 * th:512 * (th + 1)],
                                start=(ci == 0), stop=(ci == CT - 1))
                    qt = qkp.tile([128, T], F16, name=f"q_{b}_{m}", tag="q")
                    nc.vector.tensor_scalar(out=qt[:], in0=q_ps[:],
                                            scalar1=qb_sb[:, m:m + 1],
                                            scalar2=None, op0=AL.add)
                    q_sb[m] = qt
                for m in range(CT):
                    k_ps = psp.tile([128, T], F32, name=f"kps_{b}_{m}", tag="sc")
                    for th in range(TH):
                        for ci in range(CT):
                            nc.tensor.matmul(
                                k_ps[:, 512 * th:512 * (th + 1)],
                                wk_sb[:, ci, 128 * m:128 * (m + 1)],
                                xn[ci][:, 512 * th:512 * (th + 1)],
                                start=(ci == 0), stop=(ci == CT - 1))
                    kt = qkp.tile([128, S], F16, name=f"k_{b}_{m}", tag="k")
                    nc.vector.tensor_scalar(out=kt[:, 0:T], in0=k_ps[:],
                                            scalar1=kb_sb[:, m:m + 1],
                                            scalar2=None, op0=AL.add)
                    k_sb[m] = kt
                # cond k: cols T..S of k tiles
                for m in range(CT):
                    kc_ps = psp.tile([128, L], F32, name=f"kcps_{b}_{m}", tag="a")
                    for ci in range(CCT):
                        nc.tensor.matmul(kc_ps[:],
                                         wkc_sb[:, ci, 128 * m:128 * (m + 1)],
                                         c_sb[b][:, ci, :],
                                         start=(ci == 0), stop=(ci == CCT - 1))
                    nc.vector.tensor_scalar(out=k_sb[m][:, T:S], in0=kc_ps[:],
                                            scalar1=kcb_sb[:, m:m + 1],
                                            scalar2=None, op0=AL.add)

                # vT: [key-pos, head-interleaved 8*65 cols (64 vals + ones)]
                vT = {}
                for s in range(NSC):
                    sdim = 128 if s < 8 else L
                    vt = vtp.tile([sdim, HEADS * 65], F16, name=f"vt_{b}_{s}",
                                  tag="vt", padded_shape=[128, HEADS * 65])
                    for hh in range(2):  # head quads: heads 4*hh..4*hh+3
                        pv = psp.tile([sdim, 256], F32, name=f"pv_{b}_{s}_{hh}",
                                      tag="a")
                        if s < 8:
                            for ci in range(CT):
                                nc.tensor.matmul(
                                    pv[:], xn[ci][:, 128 * s:128 * (s + 1)],
                                    wv_sb[:, ci, 256 * hh:256 * (hh + 1)],
                                    start=(ci == 0), stop=(ci == CT - 1))
                        else:
                            for ci in range(CCT):
                                nc.tensor.matmul(
                                    pv[:], c_sb[b][:, ci, :],
                                    wvc_sb[:, ci, 256 * hh:256 * (hh + 1)],
                                    start=(ci == 0), stop=(ci == CCT - 1))
                        vt_view = vt.rearrange("p (h c) -> p h c", c=65)
                        pv_view = pv.rearrange("p (h c) -> p h c", c=64)
                        nc.vector.tensor_copy(
                            vt_view[:, 4 * hh:4 * (hh + 1), 0:64], pv_view[:])
                        if apply_vbias:
                            nc.vector.tensor_tensor(
                                out=vt_view[:, 4 * hh:4 * (hh + 1), 0:64],
                                in0=vt_view[:, 4 * hh:4 * (hh + 1), 0:64],
                                in1=vb_bc.rearrange("p (h c) -> p h c", c=65)[
                                    :sdim, 4 * hh:4 * (hh + 1), 0:64],
                                op=AL.add)
                        nc.vector.memset(
                            vt_view[:, 4 * hh:4 * (hh + 1), 64:65], 1.0)
                    vT[s] = vt

                # apair: proj input tiles [128, T] (2 heads stacked)
                apair = {}
                for m in range(CT):
                    apair[m] = app.tile([128, T], F16, name=f"ap_{b}_{m}", tag="apair")

                # ---- attention, head pairs -------------------------------
                for hp in range(CT):
                    hg0, hg1 = 2 * hp, 2 * hp + 1
                    pt_e, pt_o = {}, {}
                    a_e = psp.tile([65, T], F32, name=f"ae_{b}_{hp}", tag="a")
                    a_o = psp.tile([65, T], F32, name=f"ao_{b}_{hp}", tag="a")
                    for s in range(NSC):
                        sdim = 128 if s < 8 else L
                        ssl = slice(128 * s, 128 * s + sdim) if s < 8 else slice(T, S)
                        sc_e = psp.tile([sdim, T], F32, name=f"sce_{b}_{hp}_{s}",
                                        tag="sc", padded_shape=[128, T])
                        sc_o = psp.tile([sdim, T], F32, name=f"sco_{b}_{hp}_{s}",
                                        tag="sc", padded_shape=[128, T])
                        for th in range(TH):
                            tsl = slice(512 * th, 512 * (th + 1))
                            nc.tensor.matmul(sc_e[:, tsl], k_sb[hp][0:64, ssl],
                                             q_sb[hp][0:64, tsl],
                                             start=True, stop=True,
                                             tile_position=(0, 0))
                            nc.tensor.matmul(sc_o[:, tsl], k_sb[hp][64:128, ssl],
                                             q_sb[hp][64:128, tsl],
                                             start=True, stop=True,
                                             tile_position=(64, 0))
                        pe = ptp.tile([sdim, T], F16, name=f"pte_{b}_{hp}_{s}",
                                      tag="pt", padded_shape=[128, T])
                        po = ptp.tile([sdim, T], F16, name=f"pto_{b}_{hp}_{s}",
                                      tag="pt", padded_shape=[128, T])
                        nc.scalar.activation(pe[:], sc_e[:], AF.Exp)
                        nc.scalar.activation(po[:], sc_o[:], AF.Exp)
                        pt_e[s], pt_o[s] = pe, po
                        # value accumulation for this s-chunk
                        for th in range(TH):
                            tsl = slice(512 * th, 512 * (th + 1))
                            nc.tensor.matmul(a_e[:, tsl],
                                             vT[s][:, 65 * hg0:65 * hg0 + 65],
                                             pt_e[s][:, tsl],
                                             start=(s == 0), stop=(s == NSC - 1))
                            nc.tensor.matmul(a_o[:, tsl],
                                             vT[s][:, 65 * hg1:65 * hg1 + 65],
                                             pt_o[s][:, tsl],
                                             start=(s == 0), stop=(s == NSC - 1))
                    # normalize: row 64 holds the softmax denominator
                    for parity, a_ps in ((0, a_e), (1, a_o)):
                        rrow = nmp.tile([65, T], F16, name=f"rr_{b}_{hp}_{parity}",
                                        tag="rrow")
                        with nc.allow_low_precision("softmax denom recip in fp16"):
                            nc.vector.reciprocal(rrow[64:65, :], a_ps[64:65, :])
                        rbc_ps = psp.tile([64, T], F32,
                                          name=f"rbps_{b}_{hp}_{parity}", tag="sc")
                        for th in range(TH):
                            tsl = slice(512 * th, 512 * (th + 1))
                            nc.tensor.matmul(rbc_ps[:, tsl], ones16[64:65, :],
                                             rrow[64:65, tsl],
                                             start=True, stop=True)
                        rbc = nmp.tile([64, T], F16, name=f"rb_{b}_{hp}_{parity}",
                                       tag="rbc")
                        nc.vector.tensor_copy(rbc[:], rbc_ps[:])
                        if parity == 0:
                            nc.vector.tensor_tensor(out=apair[hp][0:64, :],
                                                    in0=a_ps[0:64, :], in1=rbc[:],
                                                    op=AL.mult)
                        else:
                            aodd = nmp.tile([64, T], F16, name=f"aod_{b}_{hp}",
                                            tag="aodd")
                            nc.vector.tensor_tensor(out=aodd[:], in0=a_ps[0:64, :],
                                                    in1=rbc[:], op=AL.mult)
                            nc.sync.dma_start(apair[hp][64:128, :], aodd[:])

                # ---- projection + residual -------------------------------
                for m in range(CT):
                    h_ps = psp.tile([128, T], F32, name=f"hps_{b}_{m}", tag="a")
                    for th in range(TH):
                        tsl = slice(512 * th, 512 * (th + 1))
                        for ci in range(CT):
                            nc.tensor.matmul(h_ps[:, tsl],
                                             wp_sb[:, ci, 128 * m:128 * (m + 1)],
                                             apair[ci][:, tsl],
                                             start=(ci == 0), stop=(ci == CT - 1))
                    nc.vector.tensor_tensor(out=x_sb[(b, m)][:], in0=h_ps[:],
                                            in1=x_sb[(b, m)][:], op=AL.add)
                    if apply_pbias:
                        nc.vector.tensor_scalar(out=x_sb[(b, m)][:],
                                                in0=x_sb[(b, m)][:],
                                                scalar1=pb_sb[:, m:m + 1],
                                                scalar2=None, op0=AL.add)
                    nc.sync.dma_start(outd.ap()[b, 128 * m:128 * (m + 1), :],
                                      x_sb[(b, m)][:])

    split_multi_waits(nc)
    return nc


def _prepare(inputs):
    x = np.asarray(inputs["x"], np.float32).reshape(B, C, T)
    c = np.asarray(inputs["c"], np.float32)
    gamma = np.asarray(inputs["gamma"], np.float32)
    beta = np.asarray(inputs["beta"], np.float32)
    w_qkv = np.asarray(inputs["w_qkv"], np.float32)
    b_qkv = np.asarray(inputs["b_qkv"], np.float32)
    w_c = np.asarray(inputs["w_c"], np.float32)
    b_c = np.asarray(inputs["b_c"], np.float32)
    w_p = np.asarray(inputs["w_p"], np.float32)
    b_p = np.asarray(inputs["b_p"], np.float32)

    scale = 1.0 / np.sqrt(HC)  # 0.125, exact in fp
    wq = w_qkv[0:C] * gamma[None, :]
    wk = w_qkv[C:2 * C] * gamma[None, :] * scale
    wv = w_qkv[2 * C:3 * C] * gamma[None, :]
    qb = w_qkv[0:C] @ beta + b_qkv[0:C]
    kb = (w_qkv[C:2 * C] @ beta + b_qkv[C:2 * C]) * scale
    vb = w_qkv[2 * C:3 * C] @ beta + b_qkv[2 * C:3 * C]
    wkc = w_c[0:C] * scale
    kcb = b_c[0:C] * scale
    wvc = w_c[C:2 * C]
    vcb = b_c[C:2 * C]

    def colsplit(v):  # [512] -> [128, 4] per-channel-tile columns
        return np.ascontiguousarray(v.reshape(CT, 128).T.astype(np.float32))

    # group-indicator matrices for GN stats + expansion
    G = np.zeros((128, CT, GROUPS), np.float32)
    GT = np.zeros((GROUPS, CT, 128), np.float32)
    for m in range(CT):
        for p in range(128):
            g = (m * 128 + p) // GSIZE
            G[p, m, g] = 1.0
            GT[g, m, p] = 1.0

    # vbias row in vT layout (64 values + 0 for the ones column) per head
    vbrow = np.zeros((1, HEADS * 65), np.float32)
    for h in range(HEADS):
        vbrow[0, 65 * h:65 * h + 64] = vb[64 * h:64 * (h + 1)]
    # NOTE: vcb applies to cond rows of vT with the same column layout; the
    # kernel uses a single broadcast row, so require vb == vcb when nonzero.
    apply_vbias = bool(np.any(vb != 0) or np.any(vcb != 0))
    if apply_vbias and not np.allclose(vb, vcb):
        raise NotImplementedError("distinct self/cond v biases not supported")
    apply_pbias = bool(np.any(b_p != 0))

    shared = {
        "wqT": np.ascontiguousarray(wq.T).astype(np.float16),
        "wkT": np.ascontiguousarray(wk.T).astype(np.float16),
        "wvT": np.ascontiguousarray(wv.T).astype(np.float16),
        "wkcT": np.ascontiguousarray(wkc.T).astype(np.float16),
        "wvcT": np.ascontiguousarray(wvc.T).astype(np.float16),
        "wpT": np.ascontiguousarray(w_p.T).astype(np.float16),
        "G": G, "GT": GT,
        "qb": colsplit(qb), "kb": colsplit(kb), "kcb": colsplit(kcb),
        "pb": colsplit(b_p),
        "vbrow": vbrow.astype(np.float16),
    }
    in_maps = []
    c16 = c.astype(np.float16)
    for core in range(N_CORES):
        m = dict(shared)
        m["x_sh"] = np.ascontiguousarray(x[BPC * core:BPC * (core + 1)])
        m["c_sh"] = np.ascontiguousarray(c16[BPC * core:BPC * (core + 1)])
        in_maps.append(m)
    return in_maps, apply_vbias, apply_pbias


def run(inputs, trace=False):
    in_maps, avb, apb = _prepare(inputs)
    key = (avb, apb)
    if key not in _CACHE:
        _CACHE[key] = build_program(apply_vbias=avb, apply_pbias=apb)
    nc = _CACHE[key]
    res = run_bass_kernel_spmd(nc, in_maps, core_ids=list(range(N_CORES)),
                               trace=trace)
    out = np.concatenate([res.results[c]["out"] for c in range(N_CORES)], axis=0)
    return out.reshape(B, C, HS, WS).astype(np.float32), res


def kernel(**inputs):
    out, _ = run(inputs, trace=False)
    return out


# revision 12
# speedup vs baseline: 177.6256x; 177.6256x over previous
"""Self-contained Trainium2 Bass kernel for nn_AttentionBlock (GroupNorm +
self/cross attention + projection + residual), data-parallel over batch on
8 NeuronCores.

kernel(**inputs) takes the FULL unsharded inputs of reference.setup_inputs()
and returns the FULL [16, 512, 32, 32] fp32 output.

Design notes:
- Data-parallel: 16 batch items / 8 cores = 2 per core; weights replicated;
  no collectives.
- Host prep folds the GroupNorm affine (gamma/beta) and the 1/sqrt(64)
  softmax scale into the projection weights; all matmul operands are fp16
  (PSUM accumulation stays fp32). Measured end-to-end rel err ~8e-5.
- Attention computes scores TRANSPOSED ([key_pos, t]) so the softmax
  numerator exp() needs no transposes; no max-subtraction (logits ~N(0,1),
  |logit| < ~7, exp fits fp16 easily). The value matmul contracts over key
  positions with a ones-column appended to v^T, so row 64 of the accumulator
  is the softmax denominator; normalization multiplies by its reciprocal
  broadcast via a K=1 PE outer product.
- Head pairs are row-packed on the PE (K=64 at tile_position rows 0/64).
- The per-(pair, key-chunk) stream is software-pipelined: value matmuls lag
  scores/exp by 2 chunks so the in-order PE never waits on the current exp.
"""
import numpy as np
import concourse.bass as bass
import concourse.mybir as mybir
import concourse.tile as tile
from concourse.bass_utils import run_bass_kernel_spmd

B, C, HS, WS = 16, 512, 32, 32
T = HS * WS          # 1024 spatial positions
HEADS, HC = 8, 64
L = 77               # cond sequence length
CD = 768             # cond dim
S = T + L            # 1101 key positions
GROUPS = 32
GSIZE = C // GROUPS  # 16 channels per group
EPS = 1e-5
N_CORES = 8
BPC = B // N_CORES   # batch items per core
CT = C // 128        # 4 channel tiles
CCT = CD // 128      # 6 cond channel tiles
NSC = 9              # key chunks: 8 x 128 + 77
TH = 2               # t halves of 512

F16 = mybir.dt.float16
F32 = mybir.dt.float32
AF = mybir.ActivationFunctionType
AL = mybir.AluOpType

_CACHE = {}


def split_multi_waits(nc):
    """walrus in this container accepts at most one sync wait per
    instruction; hoist extra waits onto preceding NOPs on the same engine."""
    n_split = 0
    for f in nc.m.functions:
        for blk in f.blocks:
            new_insts = []
            for inst in blk.instructions:
                si = inst.sync_info
                if si is not None and si.on_wait is not None and len(si.on_wait) > 1:
                    waits = list(si.on_wait)
                    for w in waits[:-1]:
                        nop = mybir.InstNoOp(
                            name=f"{inst.name}-wsplit{n_split}",
                            ins=[], outs=[],
                            sync_info=mybir.SyncInfo(on_wait=[w], on_update=[]),
                        )
                        nop.engine = inst.engine
                        new_insts.append(nop)
                        n_split += 1
                    si.on_wait = [waits[-1]]
                    inst.sync_info = si
                new_insts.append(inst)
            blk.instructions = new_insts
    return n_split


def build_program(apply_vbias=False, apply_pbias=False, repeat=1):
    nc = bass.Bass("TRN2", target_bir_lowering=False, debug=False, num_devices=1)

    xd = nc.dram_tensor("x_sh", [BPC, C, T], F32, kind="ExternalInput")
    cd = nc.dram_tensor("c_sh", [BPC, CD, L], F16, kind="ExternalInput")
    wqd = nc.dram_tensor("wqT", [C, C], F16, kind="ExternalInput")
    wkd = nc.dram_tensor("wkT", [C, C], F16, kind="ExternalInput")
    wvd = nc.dram_tensor("wvT", [C, C], F16, kind="ExternalInput")
    wkcd = nc.dram_tensor("wkcT", [CD, C], F16, kind="ExternalInput")
    wvcd = nc.dram_tensor("wvcT", [CD, C], F16, kind="ExternalInput")
    wpd = nc.dram_tensor("wpT", [C, C], F16, kind="ExternalInput")
    Gd = nc.dram_tensor("G", [128, CT, GROUPS], F32, kind="ExternalInput")
    GTd = nc.dram_tensor("GT", [GROUPS, CT, 128], F32, kind="ExternalInput")
    qbd = nc.dram_tensor("qb", [128, CT], F32, kind="ExternalInput")
    kbd = nc.dram_tensor("kb", [128, CT], F32, kind="ExternalInput")
    kcbd = nc.dram_tensor("kcb", [128, CT], F32, kind="ExternalInput")
    pbd = nc.dram_tensor("pb", [128, CT], F32, kind="ExternalInput")
    vbd = nc.dram_tensor("vbrow", [1, HEADS * 65], F16, kind="ExternalInput")
    outd = nc.dram_tensor("out", [BPC, C, T], F32, kind="ExternalOutput")

    with tile.TileContext(nc) as tc:
        with tc.tile_pool(name="wp", bufs=1) as wp, \
             tc.tile_pool(name="xp", bufs=8) as xp, \
             tc.tile_pool(name="gnp", bufs=8) as gnp, \
             tc.tile_pool(name="qkp", bufs=8) as qkp, \
             tc.tile_pool(name="vtp", bufs=18) as vtp, \
             tc.tile_pool(name="ptp", bufs=6) as ptp, \
             tc.tile_pool(name="app", bufs=8) as app, \
             tc.tile_pool(name="nmp", bufs=3) as nmp, \
             tc.tile_pool(name="psp", bufs=2, space="PSUM") as psp:

            # ---- first-repeat input DMAs, ahead of the (large) weight DMAs:
            # the GroupNorm stats chain gates startup; weights aren't needed
            # until the first q/k matmul.
            x_sb = {}
            c_sb = {}

            def emit_input_dmas(rep):
                for b in range(BPC):
                    for m in range(CT):
                        t_ = xp.tile([128, T], F32, name=f"x_{rep}_{b}_{m}", tag="x")
                        nc.sync.dma_start(t_[:], xd.ap()[b, 128 * m:128 * (m + 1), :])
                        x_sb[(b, m)] = t_
                    t_ = gnp.tile([128, CCT, L], F16, name=f"c_{rep}_{b}", tag="c")
                    nc.sync.dma_start(t_[:],
                                      cd.ap()[b].rearrange("(a p) l -> p a l", p=128))
                    c_sb[b] = t_

            emit_input_dmas(0)

            # ---- weights & constants -------------------------------------
            wq_sb = wp.tile([128, CT, C], F16, name="wq_sb")
            wk_sb = wp.tile([128, CT, C], F16, name="wk_sb")
            wv_sb = wp.tile([128, CT, C], F16, name="wv_sb")
            wkc_sb = wp.tile([128, CCT, C], F16, name="wkc_sb")
            wvc_sb = wp.tile([128, CCT, C], F16, name="wvc_sb")
            wp_sb = wp.tile([128, CT, C], F16, name="wp_sb")
            nc.sync.dma_start(wq_sb[:], wqd.ap().rearrange("(a p) o -> p a o", p=128))
            nc.sync.dma_start(wk_sb[:], wkd.ap().rearrange("(a p) o -> p a o", p=128))
            nc.sync.dma_start(wv_sb[:], wvd.ap().rearrange("(a p) o -> p a o", p=128))
            nc.sync.dma_start(wkc_sb[:], wkcd.ap().rearrange("(a p) o -> p a o", p=128))
            nc.sync.dma_start(wvc_sb[:], wvcd.ap().rearrange("(a p) o -> p a o", p=128))
            nc.sync.dma_start(wp_sb[:], wpd.ap().rearrange("(a p) o -> p a o", p=128))
            G_sb = wp.tile([128, CT, GROUPS], F32, name="G_sb")
            GT_sb = wp.tile([GROUPS, CT, 128], F32, name="GT_sb")
            nc.sync.dma_start(G_sb[:], Gd.ap())
            nc.sync.dma_start(GT_sb[:], GTd.ap())
            qb_sb = wp.tile([128, CT], F32, name="qb_sb")
            kb_sb = wp.tile([128, CT], F32, name="kb_sb")
            kcb_sb = wp.tile([128, CT], F32, name="kcb_sb")
            pb_sb = wp.tile([128, CT], F32, name="pb_sb")
            nc.sync.dma_start(qb_sb[:], qbd.ap())
            nc.sync.dma_start(kb_sb[:], kbd.ap())
            nc.sync.dma_start(kcb_sb[:], kcbd.ap())
            nc.sync.dma_start(pb_sb[:], pbd.ap())
            ones16 = wp.tile([65, HC], F16, name="ones16")
            nc.vector.memset(ones16[:], 1.0)
            eps_sb = wp.tile([GROUPS, 1], F32, name="eps_sb")
            nc.vector.memset(eps_sb[:], EPS)
            if apply_vbias:
                vb_bc = wp.tile([128, HEADS * 65], F16, name="vb_bc")
                nc.gpsimd.dma_start(
                    out=vb_bc[:],
                    in_=bass.AP(tensor=vbd.ap().tensor, offset=0,
                                ap=[[0, 128], [1, HEADS * 65]]),
                )

            for _rep in range(repeat):
                if _rep > 0:
                    x_sb = {}
                    c_sb = {}
                    emit_input_dmas(_rep)

                # ---- GroupNorm stats for both batches (sqrts early, before
                # any exp, to avoid ACT table-set thrash) ------------------
                E_sb = {}
                for b in range(BPC):
                    s12 = {}
                    for m in range(CT):
                        st = gnp.tile([128, 2, 6], F32,
                                      name=f"bnst_{_rep}_{b}_{m}", tag="bnst")
                        for sg in range(2):
                            nc.vector.bn_stats(
                                out=st[:, sg, :],
                                in_=x_sb[(b, m)][:, 512 * sg:512 * (sg + 1)])
                        mv = gnp.tile([128, 2], F32,
                                      name=f"mv_{_rep}_{b}_{m}", tag="mv")
                        nc.vector.bn_aggr(out=mv[:], in_=st[:])
                        s12t = gnp.tile([128, 2], F32,
                                        name=f"s12_{_rep}_{b}_{m}", tag="s12")
                        # col0 = per-channel mean, col1 = per-channel E[x^2]
                        nc.vector.tensor_copy(s12t[:, 0:1], mv[:, 0:1])
                        nc.vector.tensor_scalar(
                            out=s12t[:, 1:2], in0=mv[:, 0:1], scalar1=mv[:, 0:1],
                            scalar2=None, op0=AL.mult)
                        nc.vector.tensor_tensor(
                            out=s12t[:, 1:2], in0=s12t[:, 1:2], in1=mv[:, 1:2],
                            op=AL.add)
                        s12[m] = s12t
                    st_ps = psp.tile([GROUPS, 2], F32, name=f"stps_{_rep}_{b}",
                                     tag="a")
                    for m in range(CT):
                        nc.tensor.matmul(st_ps[:], G_sb[:, m, :], s12[m][:],
                                         start=(m == 0), stop=(m == CT - 1))
                    grp = gnp.tile([GROUPS, 6], F32, name=f"grp_{_rep}_{b}",
                                   tag="grp")
                    nc.vector.tensor_copy(grp[:, 4:6], st_ps[:])
                    # cols: 0=mean 1=rstd 2=mean 3=var 4=gsum_mean 5=gsum_e2
                    nc.vector.tensor_scalar_mul(grp[:, 2:3], in0=grp[:, 4:5],
                                                scalar1=1.0 / GSIZE)
                    nc.vector.tensor_scalar_mul(grp[:, 3:4], in0=grp[:, 5:6],
                                                scalar1=1.0 / GSIZE)
                    nc.vector.tensor_scalar(out=grp[:, 1:2], in0=grp[:, 2:3],
                                            scalar1=grp[:, 2:3], scalar2=None,
                                            op0=AL.mult)
                    nc.vector.tensor_tensor(out=grp[:, 3:4], in0=grp[:, 3:4],
                                            in1=grp[:, 1:2], op=AL.subtract)
                    nc.scalar.activation(grp[:, 3:4], grp[:, 3:4], AF.Sqrt,
                                         bias=eps_sb[:])
                    nc.vector.reciprocal(grp[:, 1:2], grp[:, 3:4])
                    nc.vector.tensor_copy(grp[:, 0:1], grp[:, 2:3])
                    for m in range(CT):
                        e_ps = psp.tile([128, 2], F32,
                                        name=f"eps_{_rep}_{b}_{m}", tag="a")
                        nc.tensor.matmul(e_ps[:], GT_sb[:, m, :], grp[:, 0:2],
                                         start=True, stop=True)
                        e_t = gnp.tile([128, 2], F32,
                                       name=f"E_{_rep}_{b}_{m}", tag="E")
                        nc.vector.tensor_copy(e_t[:], e_ps[:])
                        E_sb[(b, m)] = e_t

                # ---- per-batch pipeline ----------------------------------
                for b in range(BPC):
                    # xn = (x - mean) * rstd (affine folded into weights)
                    xn = {}
                    for m in range(CT):
                        xnt = qkp.tile([128, T], F16,
                                       name=f"xn_{_rep}_{b}_{m}", tag="xn")
                        nc.vector.tensor_scalar(
                            out=xnt[:], in0=x_sb[(b, m)][:],
                            scalar1=E_sb[(b, m)][:, 0:1],
                            scalar2=E_sb[(b, m)][:, 1:2],
                            op0=AL.subtract, op1=AL.mult)
                        xn[m] = xnt

                    q_sb = {}
                    k_sb = {}

                    def emit_qk(m):
                        q_ps = psp.tile([128, T], F32,
                                        name=f"qps_{_rep}_{b}_{m}", tag="sc")
                        for th in range(TH):
                            for ci in range(CT):
                                nc.tensor.matmul(
                                    q_ps[:, 512 * th:512 * (th + 1)],
                                    wq_sb[:, ci, 128 * m:128 * (m + 1)],
                                    xn[ci][:, 512 * th:512 * (th + 1)],
                                    start=(ci == 0), stop=(ci == CT - 1))
                        qt = qkp.tile([128, T], F16,
                                      name=f"q_{_rep}_{b}_{m}", tag="q")
                        nc.vector.tensor_scalar(out=qt[:], in0=q_ps[:],
                                                scalar1=qb_sb[:, m:m + 1],
                                                scalar2=None, op0=AL.add)
                        q_sb[m] = qt
                        k_ps = psp.tile([128, T], F32,
                                        name=f"kps_{_rep}_{b}_{m}", tag="sc")
                        for th in range(TH):
                            for ci in range(CT):
                                nc.tensor.matmul(
                                    k_ps[:, 512 * th:512 * (th + 1)],
                                    wk_sb[:, ci, 128 * m:128 * (m + 1)],
                                    xn[ci][:, 512 * th:512 * (th + 1)],
                                    start=(ci == 0), stop=(ci == CT - 1))
                        kt = qkp.tile([128, S], F16,
                                      name=f"k_{_rep}_{b}_{m}", tag="k")
                        nc.vector.tensor_scalar(out=kt[:, 0:T], in0=k_ps[:],
                                                scalar1=kb_sb[:, m:m + 1],
                                                scalar2=None, op0=AL.add)
                        k_sb[m] = kt
                        kc_ps = psp.tile([128, L], F32,
                                         name=f"kcps_{_rep}_{b}_{m}", tag="a")
                        for ci in range(CCT):
                            nc.tensor.matmul(kc_ps[:],
                                             wkc_sb[:, ci, 128 * m:128 * (m + 1)],
                                             c_sb[b][:, ci, :],
                                             start=(ci == 0), stop=(ci == CCT - 1))
                        nc.vector.tensor_scalar(out=k_sb[m][:, T:S], in0=kc_ps[:],
                                                scalar1=kcb_sb[:, m:m + 1],
                                                scalar2=None, op0=AL.add)

                    for m in range(CT):
                        emit_qk(m)

                    # vT: [key_pos, 8 heads x (64 values + ones column)]
                    vT = {}
                    for s in range(NSC):
                        sdim = 128 if s < 8 else L
                        vt = vtp.tile([sdim, HEADS * 65], F16,
                                      name=f"vt_{_rep}_{b}_{s}", tag="vt",
                                      padded_shape=[128, HEADS * 65])
                        pv = psp.tile([sdim, 512], F32,
                                      name=f"pv_{_rep}_{b}_{s}", tag="a",
                                      padded_shape=[128, 512])
                        if s < 8:
                            for ci in range(CT):
                                nc.tensor.matmul(
                                    pv[:], xn[ci][:, 128 * s:128 * (s + 1)],
                                    wv_sb[:, ci, :],
                                    start=(ci == 0), stop=(ci == CT - 1))
                        else:
                            for ci in range(CCT):
                                nc.tensor.matmul(
                                    pv[:], c_sb[b][:, ci, :], wvc_sb[:, ci, :],
                                    start=(ci == 0), stop=(ci == CCT - 1))
                        vt_view = vt.rearrange("p (h c) -> p h c", c=65)
                        pv_view = pv.rearrange("p (h c) -> p h c", c=64)
                        nc.vector.tensor_copy(vt_view[:, :, 0:64], pv_view[:])
                        if apply_vbias:
                            nc.vector.tensor_tensor(
                                out=vt_view[:, :, 0:64],
                                in0=vt_view[:, :, 0:64],
                                in1=vb_bc.rearrange("p (h c) -> p h c", c=65)[
                                    :sdim, :, 0:64],
                                op=AL.add)
                        nc.vector.memset(vt_view[:, :, 64:65], 1.0)
                        vT[s] = vt

                    apair = {}
                    for m in range(CT):
                        apair[m] = app.tile([128, T], F16,
                                            name=f"ap_{_rep}_{b}_{m}", tag="apair")

                    # ---- attention: flat (pair, chunk) pipeline ----------
                    LAG = 2
                    events = [(hp, s) for hp in range(CT) for s in range(NSC)]
                    state = {}

                    def emit_scores(hp, s):
                        if s == 0:
                            a_e = psp.tile([65, T], F32,
                                           name=f"ae_{_rep}_{b}_{hp}", tag="a")
                            a_o = psp.tile([65, T], F32,
                                           name=f"ao_{_rep}_{b}_{hp}", tag="a")
                            state[hp] = {"a_e": a_e, "a_o": a_o,
                                         "pt_e": {}, "pt_o": {}}
                        st = state[hp]
                        sdim = 128 if s < 8 else L
                        ssl = slice(128 * s, 128 * s + sdim) if s < 8 else slice(T, S)
                        sc_e = psp.tile([sdim, T], F32,
                                        name=f"sce_{_rep}_{b}_{hp}_{s}",
                                        tag="sc", padded_shape=[128, T])
                        sc_o = psp.tile([sdim, T], F32,
                                        name=f"sco_{_rep}_{b}_{hp}_{s}",
                                        tag="sc", padded_shape=[128, T])
                        for th in range(TH):
                            tsl = slice(512 * th, 512 * (th + 1))
                            nc.tensor.matmul(sc_e[:, tsl], k_sb[hp][0:64, ssl],
                                             q_sb[hp][0:64, tsl],
                                             start=True, stop=True,
                                             tile_position=(0, 0))
                            nc.tensor.matmul(sc_o[:, tsl], k_sb[hp][64:128, ssl],
                                             q_sb[hp][64:128, tsl],
                                             start=True, stop=True,
                                             tile_position=(64, 0))
                        pe = ptp.tile([sdim, T], F16,
                                      name=f"pte_{_rep}_{b}_{hp}_{s}",
                                      tag="pt", padded_shape=[128, T])
                        po = ptp.tile([sdim, T], F16,
                                      name=f"pto_{_rep}_{b}_{hp}_{s}",
                                      tag="pt", padded_shape=[128, T])
                        nc.scalar.activation(pe[:], sc_e[:], AF.Exp)
                        nc.scalar.activation(po[:], sc_o[:], AF.Exp)
                        st["pt_e"][s], st["pt_o"][s] = pe, po

                    def emit_value(hp, s):
                        st = state[hp]
                        hg0, hg1 = 2 * hp, 2 * hp + 1
                        for th in range(TH):
                            tsl = slice(512 * th, 512 * (th + 1))
                            nc.tensor.matmul(st["a_e"][:, tsl],
                                             vT[s][:, 65 * hg0:65 * hg0 + 65],
                                             st["pt_e"][s][:, tsl],
                                             start=(s == 0), stop=(s == NSC - 1))
                            nc.tensor.matmul(st["a_o"][:, tsl],
                                             vT[s][:, 65 * hg1:65 * hg1 + 65],
                                             st["pt_o"][s][:, tsl],
                                             start=(s == 0), stop=(s == NSC - 1))

                    def emit_normalize(hp):
                        st = state[hp]
                        for parity, a_ps in ((0, st["a_e"]), (1, st["a_o"])):
                            rrow = nmp.tile([65, T], F16,
                                            name=f"rr_{_rep}_{b}_{hp}_{parity}",
                                            tag="rrow")
                            with nc.allow_low_precision("denom recip fp16"):
                                nc.vector.reciprocal(rrow[64:65, :],
                                                     a_ps[64:65, :])
                            rbc_ps = psp.tile([64, T], F32,
                                              name=f"rbps_{_rep}_{b}_{hp}_{parity}",
                                              tag="sc")
                            for th in range(TH):
                                tsl = slice(512 * th, 512 * (th + 1))
                                nc.tensor.matmul(rbc_ps[:, tsl],
                                                 ones16[64:65, :],
                                                 rrow[64:65, tsl],
                                                 start=True, stop=True)
                            rbc = nmp.tile([64, T], F16,
                                           name=f"rb_{_rep}_{b}_{hp}_{parity}",
                                           tag="rbc")
                            nc.vector.tensor_copy(rbc[:], rbc_ps[:])
                            if parity == 0:
                                nc.vector.tensor_tensor(
                                    out=apair[hp][0:64, :],
                                    in0=a_ps[0:64, :], in1=rbc[:], op=AL.mult)
                            else:
                                aodd = nmp.tile([64, T], F16,
                                                name=f"aod_{_rep}_{b}_{hp}",
                                                tag="aodd")
                                nc.vector.tensor_tensor(
                                    out=aodd[:], in0=a_ps[0:64, :],
                                    in1=rbc[:], op=AL.mult)
                                nc.sync.dma_start(apair[hp][64:128, :], aodd[:])
                        del state[hp]

                    for idx in range(len(events) + LAG):
                        if idx < len(events):
                            emit_scores(*events[idx])
                        if idx >= LAG:
                            hp, s = events[idx - LAG]
                            emit_value(hp, s)
                            if s == NSC - 1:
                                emit_normalize(hp)

                    # ---- projection + residual ---------------------------
                    for m in range(CT):
                        h_ps = psp.tile([128, T], F32,
                                        name=f"hps_{_rep}_{b}_{m}", tag="a")
                        for th in range(TH):
                            tsl = slice(512 * th, 512 * (th + 1))
                            for ci in range(CT):
                                nc.tensor.matmul(
                                    h_ps[:, tsl],
                                    wp_sb[:, ci, 128 * m:128 * (m + 1)],
                                    apair[ci][:, tsl],
                                    start=(ci == 0), stop=(ci == CT - 1))
                        nc.vector.tensor_tensor(out=x_sb[(b, m)][:], in0=h_ps[:],
                                                in1=x_sb[(b, m)][:], op=AL.add)
                        if apply_pbias:
                            nc.vector.tensor_scalar(out=x_sb[(b, m)][:],
                                                    in0=x_sb[(b, m)][:],
                                                    scalar1=pb_sb[:, m:m + 1],
                                                    scalar2=None, op0=AL.add)
                        nc.sync.dma_start(outd.ap()[b, 128 * m:128 * (m + 1), :],
                                          x_sb[(b, m)][:])

    split_multi_waits(nc)
    return nc


def _prepare(inputs):
    x = np.asarray(inputs["x"], np.float32).reshape(B, C, T)
    c = np.asarray(inputs["c"], np.float32)
    gamma = np.asarray(inputs["gamma"], np.float32)
    beta = np.asarray(inputs["beta"], np.float32)
    w_qkv = np.asarray(inputs["w_qkv"], np.float32)
    b_qkv = np.asarray(inputs["b_qkv"], np.float32)
    w_c = np.asarray(inputs["w_c"], np.float32)
    b_c = np.asarray(inputs["b_c"], np.float32)
    w_p = np.asarray(inputs["w_p"], np.float32)
    b_p = np.asarray(inputs["b_p"], np.float32)

    scale = 1.0 / np.sqrt(HC)  # 0.125, exact
    wq = w_qkv[0:C] * gamma[None, :]
    wk = w_qkv[C:2 * C] * gamma[None, :] * scale
    wv = w_qkv[2 * C:3 * C] * gamma[None, :]
    qb = w_qkv[0:C] @ beta + b_qkv[0:C]
    kb = (w_qkv[C:2 * C] @ beta + b_qkv[C:2 * C]) * scale
    vb = w_qkv[2 * C:3 * C] @ beta + b_qkv[2 * C:3 * C]
    wkc = w_c[0:C] * scale
    kcb = b_c[0:C] * scale
    wvc = w_c[C:2 * C]
    vcb = b_c[C:2 * C]

    def colsplit(v):  # [512] -> [128, 4] per-channel-tile columns
        return np.ascontiguousarray(v.reshape(CT, 128).T).astype(np.float32)

    G = np.zeros((128, CT, GROUPS), np.float32)
    GT = np.zeros((GROUPS, CT, 128), np.float32)
    for m in range(CT):
        for p in range(128):
            g = (m * 128 + p) // GSIZE
            G[p, m, g] = 1.0
            GT[g, m, p] = 1.0

    vbrow = np.zeros((1, HEADS * 65), np.float32)
    for h in range(HEADS):
        vbrow[0, 65 * h:65 * h + 64] = vb[64 * h:64 * (h + 1)]
    apply_vbias = bool(np.any(vb != 0) or np.any(vcb != 0))
    if apply_vbias and not np.allclose(vb, vcb):
        raise NotImplementedError("distinct self/cond v biases not supported")
    apply_pbias = bool(np.any(b_p != 0))

    shared = {
        "wqT": np.ascontiguousarray(wq.T).astype(np.float16),
        "wkT": np.ascontiguousarray(wk.T).astype(np.float16),
        "wvT": np.ascontiguousarray(wv.T).astype(np.float16),
        "wkcT": np.ascontiguousarray(wkc.T).astype(np.float16),
        "wvcT": np.ascontiguousarray(wvc.T).astype(np.float16),
        "wpT": np.ascontiguousarray(w_p.T).astype(np.float16),
        "G": G, "GT": GT,
        "qb": colsplit(qb), "kb": colsplit(kb), "kcb": colsplit(kcb),
        "pb": colsplit(b_p),
        "vbrow": vbrow.astype(np.float16),
    }
    in_maps = []
    c16 = c.astype(np.float16)
    for core in range(N_CORES):
        m = dict(shared)
        m["x_sh"] = np.ascontiguousarray(x[BPC * core:BPC * (core + 1)])
        m["c_sh"] = np.ascontiguousarray(c16[BPC * core:BPC * (core + 1)])
        in_maps.append(m)
    return in_maps, apply_vbias, apply_pbias


def run(inputs, trace=False):
    in_maps, avb, apb = _prepare(inputs)
    key = (avb, apb)
    if key not in _CACHE:
        _CACHE[key] = build_program(apply_vbias=avb, apply_pbias=apb)
    nc = _CACHE[key]
    res = run_bass_kernel_spmd(nc, in_maps, core_ids=list(range(N_CORES)),
                               trace=trace)
    out = np.concatenate([res.results[c]["out"] for c in range(N_CORES)], axis=0)
    return out.reshape(B, C, HS, WS).astype(np.float32), res


def kernel(**inputs):
    out, _ = run(inputs, trace=False)
    return out
